# revision 8
# baseline (speedup 1.0000x reference)
"""GATv2Conv GNN message-passing kernel for 8 Trainium2 NeuronCores — v2.

Single-launch design (the v1 baseline shipped a pre-gathered 28MB/core
per-edge feature stream over the ~23MB/s axon tunnel; that transfer was
~85% of the measured time):
  * Host: append self-loops, sort edges by destination, shard contiguous
    graph ranges across 8 cores balancing edge counts; send only x.T
    sharded (~1.5MB/core) plus per-edge int16 gather indices / dst-slot /
    edge_attr streams (~1.9MB/core). Pure indexing/layout, no math.
  * Device (one SPMD program, per core):
      - xl/xr tables = x_T @ [Wl;bl | Wr;br] per 128-node window (PE);
        xl shard AllGathered into a full-graph table (HBM collective).
      - per 128-edge chunk: s/gl accumulated in PSUM from dma_gather of
        xl[src] (lo/hi split gathers — the 52k-row table exceeds int16
        indexing; out-of-half indices point at guaranteed-zero padding
        rows), dma_gather of xr[dst] (local), and a rank-1 PE update for
        edge_attr*We; leaky via ACT Prelu; logits = reduce(t*att) (DVE);
        ex = exp (ACT) expanded to 64 lanes; one-hot scatter matmuls into
        per-window PSUM, skewed one group behind (v1 pipeline kept).
      - per window: softmax-normalize, accumulate per-graph h/h^2/x sums.
      - BN stats AllReduced across cores; BN-affine + residual + 2-layer
        MLP head computed on-device per core for its own <=16 graphs.
  * Host: reassemble [100, 2] output (pure indexing).
"""

import os
import numpy as np
import ml_dtypes

os.environ.setdefault("NEURON_RT_RESET_CORES", "1")
bf16 = ml_dtypes.bfloat16

P = 128
HEADS = 4
OUT_C = 16
D = 64
GSLOT = 16
GB_CHUNKS = 8  # chunks per dma_gather batch (ring limit: <2048 idxs)
NEG_SLOPE = 0.2
BN_EPS = 1e-5

_prog_cache = {}


# --------------------------------------------------------------------------
# host prep
# --------------------------------------------------------------------------

def _prep(inputs):
    x = np.asarray(inputs["x"], np.float32)
    ei = np.asarray(inputs["edge_index"], np.int32)
    ea = np.asarray(inputs["edge_attr"], np.float32)
    batch = np.asarray(inputs["batch"], np.int32)
    N, IN_C = x.shape
    E = ei.shape[1]
    G = int(batch.max()) + 1 if batch.size else 1
    G = max(G, 100) if N == 50000 else G  # fixed 100 graphs for this problem
    NC = 8
    CHX = IN_C + 1          # x | ones

    src = np.concatenate([ei[0], np.arange(N, dtype=np.int32)])
    dst = np.concatenate([ei[1], np.arange(N, dtype=np.int32)])
    eav = np.concatenate([ea[:, 0], np.ones(N, np.float32)])
    order = np.argsort(dst, kind="stable")
    ss, ds, es = src[order], dst[order], eav[order]
    ET = ss.shape[0]

    nb = np.searchsorted(batch, np.arange(G + 1))          # node range per graph
    ecnt_g = np.bincount(batch[ds], minlength=G)            # edges per dst-graph
    csum = np.cumsum(ecnt_g)
    gb = [0]
    for k in range(1, NC):
        b = int(np.searchsorted(csum, ET * k / NC))
        gb.append(min(max(b, gb[-1] + 1), G - (NC - k)))
    gb.append(G)
    gb = np.array(gb, np.int64)

    cores = []
    Wmax, CPWmax = 1, 1
    for k in range(NC):
        g0, g1 = int(gb[k]), int(gb[k + 1])
        assert g1 - g0 <= GSLOT, f"core {k} has {g1-g0} graphs > {GSLOT}"
        n0, n1 = int(nb[g0]), int(nb[g1])
        e0, e1 = np.searchsorted(ds, [n0, n1])
        nloc = n1 - n0
        W = max(1, -(-nloc // P))
        rel = ds[e0:e1] - n0
        wofs = np.searchsorted(rel, np.arange(W + 1) * P)
        wcnt = np.diff(wofs)
        CPW = max(1, int(-(-wcnt.max() // P))) if wcnt.size else 1
        Wmax = max(Wmax, W)
        CPWmax = max(CPWmax, CPW)
        cores.append(dict(g0=g0, g1=g1, n0=n0, n1=n1, e0=int(e0), e1=int(e1),
                          rel=rel, wofs=wofs))

    W, CPW = Wmax, CPWmax
    T = W * CPW
    T8 = -(-T // GB_CHUNKS) * GB_CHUNKS
    L = T8 * P
    SH = W * P                      # xl-table shard rows per core
    TAB = NC * SH
    LOSPLIT = (TAB // 2 // P) * P   # lo/hi gather split (both halves < 32768)
    assert LOSPLIT < 32768 and TAB - LOSPLIT < 32768

    n0s = np.array([c["n0"] for c in cores] + [N], np.int64)
    nlocs = np.array([c["n1"] - c["n0"] for c in cores], np.int64)
    # guaranteed-zero padding rows (xt zero cols => xl rows are zero)
    ZLO = ZHI = -1
    for k in range(NC):
        if nlocs[k] < SH:
            cand = k * SH + int(nlocs[k])
            if ZLO < 0 and cand < LOSPLIT:
                ZLO = cand
            if ZHI < 0 and cand >= LOSPLIT:
                ZHI = cand - LOSPLIT
    assert ZLO >= 0 and ZHI >= 0, (ZLO, ZHI, nlocs, SH, LOSPLIT)

    # shared weight prep
    Wl, bl = np.asarray(inputs["Wl"], np.float32), np.asarray(inputs["bl"], np.float32)
    Wr, br = np.asarray(inputs["Wr"], np.float32), np.asarray(inputs["br"], np.float32)
    We = np.asarray(inputs["We"], np.float32)
    att = np.asarray(inputs["att"], np.float32)
    wlr = np.zeros((CHX, 2 * D), np.float32)
    wlr[:IN_C, :D] = Wl
    wlr[CHX - 1, :D] = bl
    wlr[:IN_C, D:] = Wr
    wlr[CHX - 1, D:] = br
    wres = np.concatenate([np.asarray(inputs["Wres"], np.float32),
                           np.asarray(inputs["bres"], np.float32)[None, :]], 0)
    w1 = np.concatenate([np.asarray(inputs["W1"], np.float32),
                         np.asarray(inputs["b1"], np.float32)[None, :]], 0)
    w2 = np.concatenate([np.asarray(inputs["W2"], np.float32),
                         np.asarray(inputs["b2"], np.float32)[None, :]], 0)
    attrow = np.tile(att.reshape(1, D), (1, 8))             # [1, 512]
    iotarow = np.arange(P, dtype=np.float32)[None, :]       # [1, 128]
    onesr = np.ones((1, P), np.float32)
    iotacol = np.arange(P, dtype=np.float32)[:, None]       # [128, 1] f32
    werep = We[0:1, :]                                      # [1, 64]
    misc = np.zeros((D, 4), np.float32)
    misc[:, 0] = np.asarray(inputs["gamma"], np.float32)
    misc[:, 1] = np.asarray(inputs["beta"], np.float32)
    misc[:, 2] = BN_EPS

    cnt_g = (nb[1:] - nb[:-1]).astype(np.float32)

    shared = dict(
        wlr=wlr.astype(bf16), wres=wres.astype(bf16),
        attrow=attrow.astype(bf16), iotarow=iotarow.astype(bf16),
        onesr=onesr.astype(bf16),
        werep=werep.astype(bf16), w1=w1.astype(bf16), w2=w2.astype(bf16),
        misc=misc.astype(bf16),
    )

    NIW = L // 16

    def wrap16(v):
        return np.ascontiguousarray(v.astype(np.int16).reshape(-1, 16).T)

    in_maps = []
    for k in range(NC):
        c = cores[k]
        n0, n1, e0 = c["n0"], c["n1"], c["e0"]
        nloc = n1 - n0
        relc = c["rel"]
        wofs = c["wofs"]
        Wk = len(wofs) - 1

        sel = np.full(L, -1, np.int64)          # local edge position within core
        for w in range(Wk):
            cnt = wofs[w + 1] - wofs[w]
            if cnt:
                base = w * CPW * P
                sel[base:base + cnt] = wofs[w] + np.arange(cnt)
        valid = sel >= 0
        seli = np.where(valid, sel, 0)
        relv = relc[seli] if relc.size else np.zeros(L, np.int64)

        # global xl-table row per edge source, biased by -LOSPLIT so the
        # int16 sign selects the lo/hi half on device (invalid -> ZLO row)
        gsrc = ss[e0 + seli].astype(np.int64)
        kcore = np.searchsorted(n0s[1:], gsrc, side="right")
        gidx = np.where(valid, kcore * SH + (gsrc - n0s[kcore]), ZLO)
        gi16 = gidx - LOSPLIT
        assert gi16.min() >= -32768 and gi16.max() < 32768
        # invalid slots -> row SH: a zeroed extra window of xrtab, and a
        # dst id that matches no window's iota (SH = W*128 > any iotaw)
        xri = np.where(valid, relv, SH)
        idxpk = np.concatenate([wrap16(gi16), wrap16(xri)], axis=1)

        eac = np.where(valid, es[e0 + seli], 0.0)[None, :]     # [1, L]

        xt = np.zeros((CHX, W * P), np.float32)
        xt[:IN_C, :nloc] = x[n0:n1].T
        xt[IN_C, :nloc] = 1.0

        gsl = batch[n0:n1] - c["g0"]
        gslc = np.full((W * P,), -1.0, np.float32)
        gslc[:nloc] = gsl
        gslotc = gslc.reshape(W, P).T                           # [128, W]
        cnta = np.ones((W * P,), np.float32)
        cnta[:nloc] = np.maximum(cnt_g[c["g0"]:c["g1"]], 1.0)[gsl]
        cntc = cnta.reshape(W, P).T                             # [128, W]

        pk_bf = np.concatenate([
            xt.astype(bf16).ravel(), eac.astype(bf16).ravel(),
            shared["wlr"].ravel(), shared["wres"].ravel(),
            shared["attrow"].ravel(), shared["iotarow"].ravel(),
            shared["onesr"].ravel(), shared["werep"].ravel(),
            shared["w1"].ravel(), shared["w2"].ravel(),
            shared["misc"].ravel()])
        pk = np.concatenate([
            pk_bf.view(np.int16), idxpk.ravel(),
            np.ascontiguousarray(gslotc).astype(np.int16).ravel(),
            np.ascontiguousarray(cntc).astype(np.int16).ravel()])
        in_maps.append(dict(pk=pk))

    meta = dict(N=N, IN_C=IN_C, CHX=CHX, G=G, NC=NC, W=W, CPW=CPW,
                T8=T8, SH=SH, TAB=TAB, LOSPLIT=LOSPLIT, NIW=NIW,
                ZLO=ZLO, ZHI=ZHI, gb=gb, cnt_g=cnt_g)
    return meta, in_maps, shared


# --------------------------------------------------------------------------
# bass program (single launch)
# --------------------------------------------------------------------------

def _build_main(meta, leaky_mode="prelu", debug=False):
    import concourse.bacc as bacc
    import concourse.mybir as mybir
    import concourse.tile as tile

    F32 = mybir.dt.float32
    BF = mybir.dt.bfloat16
    I16 = mybir.dt.int16
    AL = mybir.AluOpType
    AF = mybir.ActivationFunctionType
    AX = mybir.AxisListType

    N = meta["N"]
    CHX, W, CPW, T8 = meta["CHX"], meta["W"], meta["CPW"], meta["T8"]
    SH, TAB, LOSPLIT, NIW = meta["SH"], meta["TAB"], meta["LOSPLIT"], meta["NIW"]
    ZLO, ZHI = meta["ZLO"], meta["ZHI"]
    NC = meta["NC"]
    NG = T8 // 8
    GS2 = 2 * GSLOT

    nc = bacc.Bacc(None, target_bir_lowering=False, debug=debug, num_devices=NC)

    L = T8 * P
    bf_sizes = [("xt", CHX * W * P), ("eac", L), ("wlr", CHX * 2 * D),
                ("wres", CHX * D), ("attrow", 8 * D), ("iotarow", P),
                ("onesr", P), ("werep", D), ("w1", (D + 1) * D),
                ("w2", (D + 1) * 2), ("misc", D * 4)]
    NBF = sum(s for _, s in bf_sizes)
    NI = 16 * 2 * NIW
    NPK = NBF + NI + 2 * P * W
    t_pk = nc.dram_tensor("pk", [NPK], I16, kind="ExternalInput")

    def bview(name, rows):
        o = 0
        for n, s in bf_sizes:
            if n == name:
                return t_pk[o:o + s].bitcast(BF).rearrange("(r n) -> r n",
                                                           r=rows)
            o += s
        raise KeyError(name)

    t_idx_v = t_pk[NBF:NBF + NI].rearrange("(r n) -> r n", r=16)
    t_gsl_v = t_pk[NBF + NI:NBF + NI + P * W].rearrange("(r n) -> r n", r=P)
    t_cnt_v = t_pk[NBF + NI + P * W:NPK].rearrange("(r n) -> r n", r=P)

    o_out = nc.dram_tensor("o_out", [2, GSLOT], F32, kind="ExternalOutput")

    xlsh = nc.dram_tensor("xlsh", [SH, P], BF)
    xltab = nc.dram_tensor("xltab", [TAB, P], BF, addr_space="Shared")
    xrtab = nc.dram_tensor("xrtab", [SH + P, P], BF)
    t_bn = nc.dram_tensor("t_bn", [P, 1], F32)
    t_bnr = nc.dram_tensor("t_bnr", [P, 1], F32, addr_space="Shared")

    with tile.TileContext(nc) as tc:
        with tc.tile_pool(name="cst", bufs=1) as cst, \
             tc.tile_pool(name="sgl", bufs=2, space="PSUM") as ps_sgl_pool, \
             tc.tile_pool(name="win", bufs=2, space="PSUM") as ps_win_pool, \
             tc.tile_pool(name="acc", bufs=1, space="PSUM") as ps_acc_pool, \
             tc.tile_pool(name="xsm", bufs=1, space="PSUM") as ps_xsm_pool, \
             tc.tile_pool(name="str", bufs=4) as strm, \
             tc.tile_pool(name="gat", bufs=2) as gatp, \
             tc.tile_pool(name="wrk", bufs=3) as wrk:

            def load_const(t, shape, dtype):
                s = cst.tile(shape, dtype, tag=t.name)
                nc.sync.dma_start(s[:], t[:])
                return s

            def load_view(view, shape, dtype, tag):
                s = cst.tile(shape, dtype, tag=tag)
                nc.sync.dma_start(s[:], view)
                return s

            xt_t = load_view(bview("xt", CHX), [CHX, W * P], BF, "xt")
            wlr_t = load_view(bview("wlr", CHX), [CHX, 2 * D], BF, "wlr")
            attrow_t = load_view(bview("attrow", 1), [1, 8 * D], BF, "attrow")
            iotarow_t = load_view(bview("iotarow", 1), [1, P], BF, "iotarow")
            onesr_t = load_view(bview("onesr", 1), [1, P], BF, "onesr")
            werep_t = load_view(bview("werep", 1), [1, D], BF, "werep")
            wres_t = load_view(bview("wres", CHX), [CHX, D], BF, "wres")
            w1_t = load_view(bview("w1", D + 1), [D + 1, D], BF, "w1")
            w2_t = load_view(bview("w2", D + 1), [D + 1, 2], BF, "w2")
            misc_b = load_view(bview("misc", D), [D, 4], BF, "miscb")
            misc_t = cst.tile([D, 4], F32, tag="misc")
            nc.vector.tensor_copy(misc_t[:], misc_b[:])
            gsl_i = load_view(t_gsl_v, [P, W], I16, "gsli")
            gsl_t = cst.tile([P, W], F32, tag="gsl")
            nc.vector.tensor_copy(gsl_t[:], gsl_i[:])
            cnt_i = load_view(t_cnt_v, [P, W], I16, "cnti")
            cnt_f = cst.tile([P, W], F32, tag="cntf")
            nc.vector.tensor_copy(cnt_f[:], cnt_i[:])
            icv_t = cst.tile([P, W], F32, tag="icv")
            nc.vector.reciprocal(icv_t[:], cnt_f[:])

            # replicate the 16-row wrapped idx streams to 128 partitions
            idxr = cst.tile([P, 2 * NIW], I16, tag="idxr")
            for r in range(8):
                nc.sync.dma_start(idxr[16 * r:16 * (r + 1), :], t_idx_v)

            # dst-slot comparand [lane, chunk] from the wrapped xri stream via
            # a slot-linear dram bounce (idxr rows 0:16 hold the raw stream)
            relbounce = nc.dram_tensor("relbounce", [T8 * P], I16)
            nc.sync.dma_start(
                relbounce[:].rearrange("(j r) -> r j", r=16),
                idxr[0:16, NIW:2 * NIW])
            relf_i = cst.tile([P, T8], I16, tag="relfi")
            nc.sync.dma_start(
                relf_i[:], relbounce[:].rearrange("(c p) -> p c", p=P))
            relf = cst.tile([P, T8], F32, tag="relf")
            nc.vector.tensor_copy(relf[:], relf_i[:])

            # build iota/identity/att constants on device (rank-1 matmuls)
            ps_c = ps_win_pool.tile([P, 4, P], F32, tag="win", name="cbuild")
            nc.tensor.matmul(ps_c[:, 0, :], onesr_t[:], iotarow_t[:],
                             start=True, stop=True, skip_group_check=True)
            iotac_t = cst.tile([P, P], F32, tag="iotac")
            nc.scalar.activation(iotac_t[:], ps_c[:, 0, :], AF.Copy)
            nc.tensor.matmul(ps_c[:, 1, 0:1], iotarow_t[:], onesr_t[:, 0:1],
                             start=True, stop=True, skip_group_check=True)
            iotacol_t = cst.tile([P, 1], F32, tag="iotacol")
            nc.scalar.activation(iotacol_t[:], ps_c[:, 1, 0:1], AF.Copy)
            id_t = cst.tile([P, P], BF, tag="identc")
            nc.vector.tensor_scalar(id_t[:], iotac_t[:], iotacol_t[:], None,
                                    AL.is_equal)
            ps_a = ps_win_pool.tile([P, 4, P], F32, tag="win", name="abuild")
            nc.tensor.matmul(ps_a[:].rearrange("p c f -> p (c f)"), onesr_t[:],
                             attrow_t[:], start=True, stop=True,
                             skip_group_check=True)
            attc_t = cst.tile([P, 8 * D], BF, tag="attc")
            nc.scalar.activation(attc_t[:], ps_a[:].rearrange("p c f -> p (c f)"),
                                 AF.Copy)

            # build gmat (graph one-hot | scaled one-hot) on device
            gmat_t = cst.tile([P, W * GS2], BF, tag="gmat")
            gmat_v = gmat_t[:].rearrange("p (w g) -> p w g", w=W)
            for w in range(W):
                nc.vector.tensor_scalar(gmat_v[:, w, 0:GSLOT],
                                        iotac_t[:, 0:GSLOT],
                                        gsl_t[:, w:w + 1], None, AL.is_equal)
                nc.vector.tensor_scalar(gmat_v[:, w, GSLOT:GS2],
                                        gmat_v[:, w, 0:GSLOT],
                                        icv_t[:, w:w + 1], None, AL.mult)

            ps_stats = ps_acc_pool.tile([P, GS2], F32, tag="stats")
            ps_xsum = ps_xsm_pool.tile([CHX, GS2], F32, tag="xsum")

            # phase B: xl/xr tables (4 windows per psum bank) + x sums
            W4 = -(-W // 4)
            for w4 in range(W4):
                nw = min(4, W - w4 * 4)
                ps_b = ps_win_pool.tile([P, 4, 2 * D], F32, tag="win",
                                        name=f"pb{w4}")
                for j in range(nw):
                    w = w4 * 4 + j
                    nc.tensor.matmul(ps_b[:, j, :],
                                     xt_t[:, w * P:(w + 1) * P], wlr_t[:],
                                     start=True, stop=True,
                                     skip_group_check=True)
                sb_xl = wrk.tile([P, 4, P], BF, tag="xlw", name=f"xlw{w4}")
                nc.scalar.activation(sb_xl[:, 0:nw, 0:D], ps_b[:, 0:nw, 0:D],
                                     AF.Copy)
                nc.scalar.activation(sb_xl[:, 0:nw, D:2 * D], ps_b[:, 0:nw, 0:D],
                                     AF.Copy)
                sb_xr = wrk.tile([P, 4, P], BF, tag="xrw", name=f"xrw{w4}")
                nc.vector.memset(sb_xr[:, :, D:P], 0.0)
                nc.scalar.activation(sb_xr[:, 0:nw, 0:D], ps_b[:, 0:nw, D:2 * D],
                                     AF.Copy)
                nc.sync.dma_start(
                    xlsh[w4 * 4 * P:w4 * 4 * P + nw * P, :].rearrange(
                        "(w p) f -> p w f", p=P),
                    sb_xl[:, 0:nw, :])
                nc.sync.dma_start(
                    xrtab[w4 * 4 * P:w4 * 4 * P + nw * P, :].rearrange(
                        "(w p) f -> p w f", p=P),
                    sb_xr[:, 0:nw, :])
                if w4 == 0:
                    sb_z = wrk.tile([P, 1, P], BF, tag="zrow")
                    nc.vector.memset(sb_z[:], 0.0)
                    nc.sync.dma_start(
                        xrtab[SH:SH + P, :].rearrange("(w p) f -> p w f", p=P),
                        sb_z[:])
                # transpose xt windows for per-graph x sums
                ps_t = ps_win_pool.tile([P, 4, P], F32, tag="win",
                                        name=f"pt{w4}")
                for j in range(nw):
                    w = w4 * 4 + j
                    nc.tensor.matmul(ps_t[:, j, 0:CHX],
                                     xt_t[:, w * P:(w + 1) * P],
                                     id_t[0:CHX, 0:CHX],
                                     start=True, stop=True,
                                     skip_group_check=True)
                sb_xn = wrk.tile([P, 4, CHX], BF, tag="xn", name=f"xn{w4}")
                nc.scalar.activation(sb_xn[:, 0:nw, :], ps_t[:, 0:nw, 0:CHX],
                                     AF.Copy)
                for j in range(nw):
                    w = w4 * 4 + j
                    nc.tensor.matmul(ps_xsum[:], sb_xn[:, j, :],
                                     gmat_v[:, w, :],
                                     start=(w == 0), stop=(w == W - 1),
                                     skip_group_check=True)

            # AllGather the xl table shard across cores
            nc.gpsimd.collective_compute(
                "AllGather", mybir.AluOpType.bypass,
                replica_groups=[list(range(NC))],
                ins=[xlsh[:].opt()], outs=[xltab[:].opt()])

            # phase C: edge loop (scatter matmuls skewed one group behind)
            win_tiles = {}
            pend = []

            def emit_scatter(gq, oh_q, msg_q, gl_q, lg_q):
                sb_exq = wrk.tile([P, 8, D], BF, tag="exq", name=f"exq{gq}")
                nc.scalar.activation(
                    sb_exq[:].rearrange("p c (h k) -> p c h k", k=OUT_C),
                    msg_q[:, :, D:D + HEADS].unsqueeze(3).to_broadcast(
                        [P, 8, HEADS, OUT_C]),
                    AF.Copy)
                nc.vector.tensor_tensor(
                    out=msg_q[:, :, 0:D], in0=gl_q[:], in1=sb_exq[:],
                    op=AL.mult)
                flush = []
                for c8 in range(8):
                    c = gq * 8 + c8
                    w = min(c // CPW, W - 1)
                    if w not in win_tiles:
                        win_tiles[w] = ps_win_pool.tile([P, D + HEADS], F32,
                                                        tag="win", name=f"win{gq}_{w}")
                    first = (c % CPW == 0) and c < W * CPW
                    last = (c == (w + 1) * CPW - 1) if w < W - 1 else (c == T8 - 1)
                    nc.tensor.matmul(win_tiles[w][:], oh_q[:, c8, :],
                                     msg_q[:, c8, :], start=first, stop=last,
                                     skip_group_check=True)
                    if last:
                        flush.append(w)
                return flush

            def do_flush(flush):
                for w in flush:
                    ps_w = win_tiles.pop(w)
                    sb_den = wrk.tile([P, HEADS], F32, tag="den", name=f"den{w}")
                    nc.vector.tensor_scalar(sb_den[:], ps_w[:, D:D + HEADS],
                                            1e-20, None, AL.add)
                    sb_rd = wrk.tile([P, HEADS], F32, tag="rd", name=f"rd{w}")
                    nc.vector.reciprocal(sb_rd[:], sb_den[:])
                    sb_hh2 = wrk.tile([P, 2 * D], BF, tag="hh2", name=f"hh2{w}")
                    nc.vector.tensor_tensor(
                        out=sb_hh2[:, 0:D].rearrange("p (h k) -> p h k", k=OUT_C),
                        in0=ps_w[:, 0:D].rearrange("p (h k) -> p h k", k=OUT_C),
                        in1=sb_rd[:].unsqueeze(2).to_broadcast([P, HEADS, OUT_C]),
                        op=AL.mult)
                    nc.scalar.activation(sb_hh2[:, D:2 * D], sb_hh2[:, 0:D],
                                         AF.Square)
                    nc.tensor.matmul(ps_stats[:], sb_hh2[:], gmat_v[:, w, :],
                                     start=(w == 0), stop=(w == W - 1),
                                     skip_group_check=True)

            iotaw_tiles = {}

            def get_iotaw(w):
                if w not in iotaw_tiles:
                    iotaw_tiles.clear()
                    t = wrk.tile([P, P], F32, tag="iow", name=f"iow{w}")
                    nc.vector.tensor_scalar(t[:], iotac_t[:], float(w * P),
                                            None, AL.add)
                    iotaw_tiles[w] = t
                return iotaw_tiles[w]

            for g in range(NG):
                # split the biased gi16 batch into lo/hi half-table indices:
                # lo = (gi+LOSPLIT)*[gi<0] + ZLO*[gi>=0]; hi = gi*[gi>=0] + ZHI*[gi<0]
                gi_sl = idxr[:, g * 64:(g + 1) * 64]
                mlo = wrk.tile([P, 64], I16, tag="mlo")
                nc.vector.tensor_scalar(mlo[:], gi_sl, 0, None, AL.is_lt)
                mhi = wrk.tile([P, 64], I16, tag="mhi")
                nc.vector.tensor_scalar(mhi[:], gi_sl, 0, None, AL.is_ge)
                ia = wrk.tile([P, 64], I16, tag="ia")
                nc.vector.tensor_scalar(ia[:], gi_sl, LOSPLIT, None, AL.add)
                ib = wrk.tile([P, 64], I16, tag="ib")
                nc.vector.tensor_tensor(out=ib[:], in0=ia[:], in1=mlo[:],
                                        op=AL.mult)
                ic = wrk.tile([P, 64], I16, tag="ic")
                nc.vector.tensor_scalar(ic[:], mhi[:], ZLO, None, AL.mult)
                lo_t = wrk.tile([P, 64], I16, tag="ilo")
                nc.vector.tensor_tensor(out=lo_t[:], in0=ib[:], in1=ic[:],
                                        op=AL.add)
                idd = wrk.tile([P, 64], I16, tag="id")
                nc.vector.tensor_tensor(out=idd[:], in0=gi_sl, in1=mhi[:],
                                        op=AL.mult)
                ie = wrk.tile([P, 64], I16, tag="ie")
                nc.vector.tensor_scalar(ie[:], mlo[:], ZHI, None, AL.mult)
                hi_t = wrk.tile([P, 64], I16, tag="ihi")
                nc.vector.tensor_tensor(out=hi_t[:], in0=idd[:], in1=ie[:],
                                        op=AL.add)

                gr_lo = gatp.tile([P, 8, P], BF, tag="grlo")
                nc.gpsimd.dma_gather(
                    out_ap=gr_lo[:], in_ap=xltab[0:LOSPLIT, :],
                    idxs_ap=lo_t[:],
                    num_idxs=1024, num_idxs_reg=1024, elem_size=P)
                gr_hi = gatp.tile([P, 8, P], BF, tag="grhi")
                nc.gpsimd.dma_gather(
                    out_ap=gr_hi[:], in_ap=xltab[LOSPLIT:TAB, :],
                    idxs_ap=hi_t[:],
                    num_idxs=1024, num_idxs_reg=1024, elem_size=P)
                gr_xr = gatp.tile([P, 8, P], BF, tag="grxr")
                nc.gpsimd.dma_gather(
                    out_ap=gr_xr[:], in_ap=xrtab[:],
                    idxs_ap=idxr[:, NIW + g * 64:NIW + (g + 1) * 64],
                    num_idxs=1024, num_idxs_reg=1024, elem_size=P)
                ea_g = strm.tile([1, 8 * P], BF, tag="eag")
                nc.sync.dma_start(ea_g[:],
                                  bview("eac", 1)[:, g * 8 * P:(g + 1) * 8 * P])

                ps_sgl = ps_sgl_pool.tile([P, 8, 2 * D], F32, tag="sgl")
                sglf = ps_sgl[:].rearrange("p c f -> p (c f)")
                for h in range(2):
                    hs = slice(h * 512, (h + 1) * 512)
                    nc.tensor.matmul(sglf[:, hs], id_t[:],
                                     gr_lo[:].rearrange("p c f -> p (c f)")[:, hs],
                                     start=True, stop=True, skip_group_check=True)
                    nc.tensor.matmul(sglf[:, hs], id_t[:],
                                     gr_hi[:].rearrange("p c f -> p (c f)")[:, hs],
                                     start=False, stop=True, skip_group_check=True)
                    nc.tensor.matmul(sglf[:, hs], id_t[:],
                                     gr_xr[:].rearrange("p c f -> p (c f)")[:, hs],
                                     start=False, stop=True, skip_group_check=True)
                for c8 in range(8):
                    nc.tensor.matmul(ps_sgl[:, c8, 0:D],
                                     ea_g[:, c8 * P:(c8 + 1) * P], werep_t[:],
                                     start=False, stop=True,
                                     skip_group_check=True)

                sb_t = wrk.tile([P, 8, D], BF, tag="t")
                if leaky_mode == "prelu":
                    nc.scalar.activation(sb_t[:], ps_sgl[:, :, 0:D], AF.Prelu,
                                         alpha=NEG_SLOPE)
                else:
                    sb_r2 = wrk.tile([P, 8, D], BF, tag="r2")
                    nc.scalar.activation(sb_r2[:], ps_sgl[:, :, 0:D], AF.Relu,
                                         scale=-(1.0 - NEG_SLOPE))
                    for c8 in range(8):
                        nc.tensor.matmul(ps_sgl[:, c8, 0:D], id_t[:],
                                         sb_r2[:, c8, :],
                                         start=False, stop=True,
                                         skip_group_check=True)
                    nc.scalar.activation(sb_t[:], ps_sgl[:, :, 0:D], AF.Copy)
                if pend:
                    _, _, pmsg, _, plg = pend[-1]
                    nc.scalar.activation(pmsg[:, :, D:D + HEADS], plg[:], AF.Exp)
                sb_gl = wrk.tile([P, 8, D], BF, tag="gl")
                nc.scalar.activation(sb_gl[:], ps_sgl[:, :, D:2 * D], AF.Copy)

                sb_u = wrk.tile([P, 8, D], BF, tag="u")
                nc.vector.tensor_tensor(
                    out=sb_u[:], in0=sb_t[:],
                    in1=attc_t[:].rearrange("p (c f) -> p c f", c=8),
                    op=AL.mult)
                sb_lg = wrk.tile([P, 8, HEADS], F32, tag="lg")
                nc.vector.tensor_reduce(
                    out=sb_lg[:],
                    in_=sb_u[:].rearrange("p c (h k) -> p c h k", k=OUT_C),
                    axis=AX.X, op=AL.add)
                sb_msg = wrk.tile([P, 8, D + HEADS], BF, tag="msg")

                oh_t = wrk.tile([P, 8, P], BF, tag="oh")
                for c8 in range(8):
                    c = g * 8 + c8
                    iow = get_iotaw(min(c // CPW, W - 1))
                    nc.vector.tensor_scalar(
                        oh_t[:, c8, :], iow[:],
                        relf[:, c:c + 1], None, AL.is_equal)

                pend.append((g, oh_t, sb_msg, sb_gl, sb_lg))
                if len(pend) > 1:
                    do_flush(emit_scatter(*pend.pop(0)))

            while pend:
                _, _, pmsg, _, plg = pend[0]
                nc.scalar.activation(pmsg[:, :, D:D + HEADS], plg[:], AF.Exp)
                do_flush(emit_scatter(*pend.pop(0)))

            # phase D: BN stats AllReduce + on-device tail
            sb_sloc = wrk.tile([P, 1], F32, tag="sloc")
            nc.vector.tensor_reduce(out=sb_sloc[:], in_=ps_stats[:, 0:GSLOT],
                                    axis=AX.X, op=AL.add)
            nc.sync.dma_start(t_bn[:], sb_sloc[:])
            nc.gpsimd.collective_compute(
                "AllReduce", mybir.AluOpType.add,
                replica_groups=[list(range(NC))],
                ins=[t_bn[:].opt()], outs=[t_bnr[:].opt()])
            sb_sh = wrk.tile([D, 1], F32, tag="sh")
            nc.sync.dma_start(sb_sh[:], t_bnr[0:D, :])
            sb_sh2 = wrk.tile([D, 1], F32, tag="sh2")
            nc.sync.dma_start(sb_sh2[:], t_bnr[D:2 * D, :])

            mu = wrk.tile([D, 1], F32, tag="mu")
            nc.scalar.activation(mu[:], sb_sh[:], AF.Copy, scale=1.0 / N)
            e2 = wrk.tile([D, 1], F32, tag="e2")
            nc.scalar.activation(e2[:], sb_sh2[:], AF.Copy, scale=1.0 / N)
            mu2 = wrk.tile([D, 1], F32, tag="mu2")
            nc.scalar.activation(mu2[:], mu[:], AF.Square)
            var = wrk.tile([D, 1], F32, tag="var")
            nc.vector.tensor_tensor(out=var[:], in0=e2[:], in1=mu2[:],
                                    op=AL.subtract)
            sd = wrk.tile([D, 1], F32, tag="sd")
            nc.scalar.activation(sd[:], var[:], AF.Sqrt, bias=misc_t[:, 2:3])
            rsd = wrk.tile([D, 1], F32, tag="rsd")
            nc.vector.reciprocal(rsd[:], sd[:])
            A = wrk.tile([D, 1], F32, tag="A")
            nc.vector.tensor_tensor(out=A[:], in0=misc_t[:, 0:1], in1=rsd[:],
                                    op=AL.mult)
            tmp2 = wrk.tile([D, 1], F32, tag="tmp2")
            nc.vector.tensor_tensor(out=tmp2[:], in0=A[:], in1=mu[:], op=AL.mult)
            B = wrk.tile([D, 1], F32, tag="B")
            nc.vector.tensor_tensor(out=B[:], in0=misc_t[:, 1:2], in1=tmp2[:],
                                    op=AL.subtract)

            hdiv = wrk.tile([D, GSLOT], F32, tag="hdiv")
            nc.scalar.activation(hdiv[:], ps_stats[0:D, GSLOT:GS2], AF.Copy)
            pooled = wrk.tile([D, GSLOT], F32, tag="pooled")
            nc.vector.tensor_scalar(pooled[:], hdiv[:], A[:], B[:],
                                    AL.mult, AL.add)
            sb_xdiv = wrk.tile([CHX, GSLOT], BF, tag="xdiv")
            nc.scalar.activation(sb_xdiv[:], ps_xsum[:, GSLOT:GS2], AF.Copy)
            ps_res = ps_sgl_pool.tile([D, GSLOT], F32, tag="sgl")
            nc.tensor.matmul(ps_res[:], wres_t[:], sb_xdiv[:], start=True,
                             stop=True, skip_group_check=True)
            zr = wrk.tile([D + 1, GSLOT], BF, tag="zr")
            nc.vector.memset(zr[D:D + 1, :], 1.0)
            nc.vector.tensor_tensor(out=zr[0:D, :], in0=pooled[:], in1=ps_res[:],
                                    op=AL.add)
            ps_z = ps_sgl_pool.tile([D, GSLOT], F32, tag="sgl")
            nc.tensor.matmul(ps_z[:], w1_t[:], zr[:], start=True, stop=True,
                             skip_group_check=True)
            z2 = wrk.tile([D + 1, GSLOT], BF, tag="z2")
            nc.vector.memset(z2[D:D + 1, :], 1.0)
            nc.scalar.activation(z2[0:D, :], ps_z[:], AF.Relu)
            ps_o = ps_sgl_pool.tile([2, GSLOT], F32, tag="sgl")
            nc.tensor.matmul(ps_o[:], w2_t[:], z2[:], start=True, stop=True,
                             skip_group_check=True)
            sb_o = wrk.tile([2, GSLOT], F32, tag="out")
            nc.scalar.activation(sb_o[:], ps_o[:], AF.Copy)
            nc.sync.dma_start(o_out[:], sb_o[:])

    nc.compile()
    return nc


# --------------------------------------------------------------------------
# cached PJRT runner (same bass_exec custom-call stack as
# bass_utils.run_bass_kernel_spmd under axon, but the jitted executable is
# built once and reused — run_bass_kernel_spmd rebuilds the jit closure per
# call, which re-runs ~0.7s of client-side BIR compilation every launch)
# --------------------------------------------------------------------------

def _make_runner(nc, n_cores):
    import jax
    import numpy as np
    from jax.sharding import Mesh, PartitionSpec
    from jax.experimental.shard_map import shard_map
    import concourse.mybir as mybir
    from concourse import bass2jax

    bass2jax.install_neuronx_cc_hook()
    partition_name = (nc.partition_id_tensor.name
                      if nc.partition_id_tensor else None)
    in_names = []
    out_names = []
    out_avals = []
    zero_shapes = []
    for alloc in nc.m.functions[0].allocations:
        if not isinstance(alloc, mybir.MemoryLocationSet):
            continue
        name = alloc.memorylocations[0].name
        if alloc.kind == "ExternalInput":
            if name != partition_name:
                in_names.append(name)
        elif alloc.kind == "ExternalOutput":
            out_names.append(name)
            shape = tuple(alloc.tensor_shape)
            dtype = mybir.dt.np(alloc.dtype)
            out_avals.append(jax.core.ShapedArray(shape, dtype))
            zero_shapes.append((shape, dtype))
    n_params = len(in_names)
    n_outs = len(out_avals)
    bind_names = list(in_names) + out_names
    if partition_name is not None:
        bind_names.append(partition_name)

    def _body(*args):
        operands = list(args)
        if partition_name is not None:
            operands.append(bass2jax.partition_id_tensor())
        outs = bass2jax._bass_exec_p.bind(
            *operands,
            out_avals=tuple(out_avals),
            in_names=tuple(bind_names),
            out_names=tuple(out_names),
            lowering_input_output_aliases=(),
            sim_require_finite=True,
            sim_require_nnan=True,
            nc=nc,
        )
        return tuple(outs)

    donate = tuple(range(n_params, n_params + n_outs))
    devices = jax.devices()[:n_cores]
    mesh = Mesh(np.asarray(devices), ("core",))
    in_specs = (PartitionSpec("core"),) * (n_params + n_outs)
    out_specs = (PartitionSpec("core"),) * len(out_names)
    sharded = jax.jit(
        shard_map(_body, mesh=mesh, in_specs=in_specs, out_specs=out_specs,
                  check_rep=False),
        donate_argnums=donate, keep_unused=True)

    concat_zeros = [np.zeros((n_cores * s[0], *s[1:]), d) for s, d in zero_shapes]

    def run(concat_in):
        out_arrs = sharded(*concat_in, *concat_zeros)
        return [
            {name: np.asarray(out_arrs[i]).reshape(n_cores, *out_avals[i].shape)[c]
             for i, name in enumerate(out_names)}
            for c in range(n_cores)]

    def prep_in(in_maps):
        return [np.concatenate([np.asarray(m[name]) for m in in_maps], axis=0)
                for name in in_names]

    return run, prep_in


# --------------------------------------------------------------------------
# entry point
# --------------------------------------------------------------------------

def kernel(**inputs):
    meta, in_maps, shared = _prep(inputs)
    key = ("main2", meta["CHX"], meta["W"], meta["CPW"], meta["T8"],
           meta["N"], meta["ZLO"], meta["ZHI"], _LEAKY_MODE)
    if key not in _prog_cache:
        _prog_cache[key] = _build_main(meta, leaky_mode=_LEAKY_MODE,
                                       debug=(_RUN_MODE == "sim"))
    nc_main = _prog_cache[key]

    NC = meta["NC"]
    core_ids = list(range(NC))
    global LAST_EXEC_NS
    if _RUN_MODE == "sim":
        from concourse.bass_interp import MultiCoreSim
        sim = MultiCoreSim(nc_main, num_cores=NC)
        for k in range(NC):
            for n, a in in_maps[k].items():
                sim.cores[k].tensor(n)[:] = a
        sim.simulate()
        res1 = [{"o_out": np.array(sim.cores[k].tensor("o_out"))}
                for k in range(NC)]
        LAST_EXEC_NS = [None]
    else:
        import time as _time
        rkey = ("runner",) + key
        if rkey not in _prog_cache:
            # first call: compile + run via run_bass_kernel_spmd, then build
            # the cached jit runner for subsequent calls
            from concourse.bass_utils import run_bass_kernel_spmd
            _t0 = _time.time()
            r1 = run_bass_kernel_spmd(nc_main, in_maps, core_ids, **_RUN_KW)
            _t1 = _time.time()
            res1 = r1.results
            LAST_EXEC_NS = [getattr(r1, "exec_time_ns", None)
                            or int((_t1 - _t0) * 1e9)]
            _prog_cache[rkey] = _make_runner(nc_main, NC)
            try:
                run, prep_in = _prog_cache[rkey]
                res1 = run(prep_in(in_maps))  # one-time jit warmup
            except Exception:
                # keep the validated spmd result; rebuild the runner lazily
                del _prog_cache[rkey]
        else:
            run, prep_in = _prog_cache[rkey]
            concat_in = prep_in(in_maps)
            _t0 = _time.time()
            try:
                res1 = run(concat_in)
            except Exception:
                # transient device wedge (e.g. NRT_EXEC_UNIT_UNRECOVERABLE):
                # fall back to a fresh compile+load launch, which resets cores
                from concourse.bass_utils import run_bass_kernel_spmd
                res1 = run_bass_kernel_spmd(nc_main, in_maps, core_ids,
                                            **_RUN_KW).results
            _t1 = _time.time()
            LAST_EXEC_NS = [int((_t1 - _t0) * 1e9)]

    G = meta["G"]
    gb = meta["gb"]
    out = np.zeros((G, 2), np.float32)
    for g in range(G):
        k = int(np.searchsorted(gb, g, side="right")) - 1
        slot = g - int(gb[k])
        out[g] = res1[k]["o_out"][:, slot]
    return out


_LEAKY_MODE = "prelu"
_RUN_MODE = "hw"
_RUN_KW = {}
LAST_EXEC_NS = None
